# revision 9
# baseline (speedup 1.0000x reference)
"""Trainium2 Bass kernel for nn_Attention_47682726920277.

Causal multi-head attention with RoPE:
  q/k/v = x @ w{q,k,v}.T ; RoPE(q, k) ; att = softmax(mask(q k^T / 8)) ; out = (att v) @ wo.T
Shapes: x [2, 2048, 1024], 16 heads of dim 64, fp32.

Sharding (8 cores): data-parallel over batch (2) x tensor-parallel over heads (4 per
core). Each core computes its 4 heads' attention and a partial out via its wo row
block; the final all-reduce is the host-side sum of the 4 partials per batch.

Per-core layout strategy:
  - Host passes x^T and the weight slices pre-transposed, as float16 (fp16 streams
    1 col/cycle through the PE array with fp32 PSUM accumulation; ~5e-4 rounding).
  - Q,K are produced transposed (QT/KT [256ch, T]) so scores come out as
    S^T [k, q] directly; softmax needs no max-subtraction (logits are small) and the
    denominator is obtained by augmenting V with a ones column (row 64 of the PV
    accumulator = sum_k exp). Exp runs on the scalar engine with the 1/8 scale fused.
  - Softmax reciprocals are batched: sums rows are staged to DRAM, one [8, T/4]
    vector reciprocal per head-pair, and re-broadcast across partitions with
    stride-0 DMA reads.
  - Emission interleaves head-pair 1's projections with head-pair 0's attention so
    the tensor engine has fill work while exp runs.
"""
import sys
import types
import numpy as np

B = 2
T = 2048
D = 1024
H = 16
HD = 64
NCORES = 8
GROUPS = NCORES // B          # head-groups per batch
HPC = H // GROUPS             # heads per core = 4
CH = HPC * HD                 # channels per core = 256
NQ = 512                      # PSUM bank width (fp32)
P = 128

_prog_cache = {}


def _install_ntff_shim():
    """The agent image's antenv lacks axon_hooks; inject it so trace=True works."""
    try:
        import antenv.axon_hooks  # noqa: F401
        return
    except ImportError:
        pass
    try:
        import trn_agent_boot.trn_boot as tb
        hook = tb._ntff_profile_via_ctypes('/opt/axon/libaxon_pjrt.so')
        if hook is None:
            return
        mod = types.ModuleType('antenv.axon_hooks')
        mod.get_axon_ntff_profile_hook = lambda: hook
        mod.set_axon_ntff_profile_hook = lambda h: None
        sys.modules['antenv.axon_hooks'] = mod
        import antenv
        antenv.axon_hooks = mod
    except Exception:
        pass


def _build_program(causal: bool):
    import concourse.bass as bass
    from concourse import bacc
    import concourse.tile as tile
    from concourse import mybir

    F32 = mybir.dt.float32
    F16 = mybir.dt.float16
    AF = mybir.ActivationFunctionType
    MUL = mybir.AluOpType.mult
    ADD = mybir.AluOpType.add

    NT = T // NQ          # proj/attention q-chunks (4)
    NKB = T // P          # k-blocks (16)
    DB = D // P           # d-blocks (8)
    CB = CH // P          # channel blocks = head-pair blocks (2)

    nc = bacc.Bacc("TRN2", target_bir_lowering=False, debug=False)

    xT = nc.dram_tensor("xT", [D, T], F16, kind="ExternalInput").ap()
    wqT = nc.dram_tensor("wqT", [D, CH], F16, kind="ExternalInput").ap()
    wkT = nc.dram_tensor("wkT", [D, CH], F16, kind="ExternalInput").ap()
    wvT = nc.dram_tensor("wvT", [D, CH], F16, kind="ExternalInput").ap()
    woT = nc.dram_tensor("woT", [CH, D], F16, kind="ExternalInput").ap()
    cosT = nc.dram_tensor("cosT", [P, T], F32, kind="ExternalInput").ap()
    sinS = nc.dram_tensor("sinS", [P, T], F16, kind="ExternalInput").ap()
    tri = nc.dram_tensor("tri", [P, P], F16, kind="ExternalInput").ap()
    onescol = nc.dram_tensor("onescol", [P, NKB * HPC], F16, kind="ExternalInput").ap()
    out = nc.dram_tensor("out", [T, D], F32, kind="ExternalOutput").ap()

    with tile.TileContext(nc) as tc:
        with tc.tile_pool(name="singles", bufs=1) as singles, \
             tc.tile_pool(name="rope16", bufs=3) as rope16, \
             tc.tile_pool(name="ptp", bufs=6) as ptp, \
             tc.tile_pool(name="obp", bufs=3) as obp, \
             tc.tile_pool(name="ssm", bufs=3) as ssm, \
             tc.tile_pool(name="clp", bufs=2) as clp, \
             tc.tile_pool(name="bcp", bufs=3) as bcp, \
             tc.tile_pool(name="dramp", bufs=1, space="DRAM") as dramp, \
             tc.tile_pool(name="st_ps", bufs=5, space="PSUM") as st_ps_pool, \
             tc.tile_pool(name="ot_ps", bufs=3, space="PSUM") as ot_ps_pool:

            # ---- resident loads ----
            xT_sb = singles.tile([P, DB, T], F16)
            wqT_sb = singles.tile([P, DB, CH], F16)
            wkT_sb = singles.tile([P, DB, CH], F16)
            wvT_sb = singles.tile([P, DB, CH], F16)
            woT_sb = singles.tile([P, CB, D], F16)
            cosT_sb = singles.tile([P, T], F32)
            sinS_sb = singles.tile([P, T], F16)
            tri_sb = singles.tile([P, P], F16)
            xTr = xT.rearrange("(o p) t -> p o t", p=P)
            nc.sync.dma_start(wqT_sb[:], wqT.rearrange("(o p) c -> p o c", p=P))
            for o in range(DB):
                nc.sync.dma_start(xT_sb[:, o, :], xTr[:, o, :])
                if o == 1:
                    nc.sync.dma_start(cosT_sb[:], cosT[:])
                    nc.sync.dma_start(sinS_sb[:], sinS[:])
                if o == 3:
                    nc.sync.dma_start(
                        wkT_sb[:], wkT.rearrange("(o p) c -> p o c", p=P))
                if o == 5:
                    nc.sync.dma_start(
                        wvT_sb[:], wvT.rearrange("(o p) c -> p o c", p=P))
            nc.sync.dma_start(woT_sb[:], woT.rearrange("(o p) c -> p o c", p=P))
            nc.sync.dma_start(tri_sb[:], tri[:])

            QT_sb = singles.tile([P, CB, T], F16)
            KT_sb = singles.tile([P, CB, T], F16)
            attnT_sb = singles.tile([P, CB, T], F16)
            # V with a ones column per head: [kb, head, 65]
            vaug = singles.tile([P, NKB, HPC, HD + 1], F16)
            nc.sync.dma_start(
                vaug[:, :, :, HD:HD + 1],
                onescol.rearrange("p (a b) -> p a b", a=NKB)[:, :, :, None])
            # unnormalized attention outputs + sums scratch
            otsb = [singles.tile([HD, 2 * NT, NQ], F32, name=f"otsb_{hp}")
                    for hp in range(CB)]
            sums_sb = [singles.tile([1, 2 * NT, NQ], F32, name=f"sums_sb_{hp}")
                       for hp in range(CB)]
            sums_dram = dramp.tile([CB, 2 * NT, NQ], F32)
            recip_dram = dramp.tile([CB, 2 * NT, NQ], F32)

            # ---- projections (one head-pair block at a time), RoPE fused ----
            def project_T(w_sb, dst_sb, cb, pname):
                for m in range(NT):
                    ps = st_ps_pool.tile([P, NQ], F32, tag="st",
                                         name=f"prj_{pname}_{cb}_{m}")
                    for o in range(DB):
                        nc.tensor.matmul(
                            ps[:],
                            w_sb[:, o, cb * P:(cb + 1) * P],
                            xT_sb[:, o, m * NQ:(m + 1) * NQ],
                            start=(o == 0), stop=(o == DB - 1))
                    cs = slice(m * NQ, (m + 1) * NQ)
                    # q' = q*cos + shift(q)*sinS  (shift = partner rows, sign in table)
                    nc.vector.tensor_tensor(dst_sb[:, cb, cs], ps[:],
                                            cosT_sb[:, cs], MUL)
                    qraw = rope16.tile([P, NQ], F16, tag="qraw",
                                       name=f"qr_{pname}_{cb}_{m}")
                    nc.vector.tensor_copy(qraw[:], ps[:])
                    tmp = rope16.tile([P, NQ], F16, tag="tmp",
                                      name=f"tm_{pname}_{cb}_{m}")
                    for g in range(4):
                        src = (g ^ 1) * 32
                        dst = g * 32
                        nc.vector.tensor_tensor(
                            tmp[dst:dst + 32, :],
                            qraw[src:src + 32, :],
                            sinS_sb[src:src + 32, cs], MUL)
                    nc.vector.tensor_tensor(dst_sb[:, cb, cs],
                                            dst_sb[:, cb, cs], tmp[:], ADD)

            def project_v():
                for i in range(NKB):
                    ps = st_ps_pool.tile([P, NQ], F32, tag="st", name=f"v_{i}")
                    vps = ps[:, :CH]
                    for o in range(DB):
                        nc.tensor.matmul(
                            vps,
                            xT_sb[:, o, i * P:(i + 1) * P],
                            wvT_sb[:, o, :],
                            start=(o == 0), stop=(o == DB - 1))
                    nc.vector.tensor_copy(
                        vaug[:, i, :, 0:HD],
                        vps.rearrange("p (h d) -> p h d", h=HPC))

            def kb_list(qc):
                return list(range(min(NKB, (qc + 1) * (NQ // P)))) if causal \
                    else list(range(NKB))

            def attention(hp, qcs=None):
                for qc in (range(NT) if qcs is None else qcs):
                    kbs = kb_list(qc)
                    q0 = qc * NQ
                    otps = [ot_ps_pool.tile([HD + 1, NQ], F32, tag="ot",
                                            name=f"ot_{hp}_{qc}_{i}")
                            for i in range(2)]

                    def finish(half, kb, stp, qsl):
                        h = hp * 2 + half
                        pt = ptp.tile([P, NQ], F16, tag="pt",
                                      name=f"pt_{hp}_{qc}_{half}_{kb}")
                        nc.scalar.activation(pt[:, qsl:NQ], stp[:, qsl:NQ],
                                             AF.Exp, scale=float(HD) ** -0.5)
                        if causal and kb * P >= q0:
                            nc.vector.tensor_tensor(
                                pt[:, qsl:qsl + P], pt[:, qsl:qsl + P],
                                tri_sb[:], MUL)
                        nc.tensor.matmul(
                            otps[half][:, qsl:NQ],
                            vaug[:, kb, h, :],
                            pt[:, qsl:NQ],
                            start=(kb == kbs[0]), stop=(kb == kbs[-1]))

                    pend = [None, None]
                    for kb in kbs:
                        qsl = max(0, kb * P - q0) if causal else 0
                        for half in range(2):
                            hb = half * HD
                            stp = st_ps_pool.tile([P, NQ], F32, tag="st",
                                                  name=f"st_{hp}_{qc}_{half}_{kb}")
                            nc.tensor.matmul(
                                stp[:, qsl:NQ],
                                KT_sb[hb:hb + HD, hp, kb * P:(kb + 1) * P],
                                QT_sb[hb:hb + HD, hp, q0 + qsl:q0 + NQ],
                                start=True, stop=True)
                            if pend[half] is not None:
                                finish(half, *pend[half])
                            pend[half] = (kb, stp, qsl)
                    for half in range(2):
                        finish(half, *pend[half])

                    # stage unnormalized output + sums (PSUM must drain promptly)
                    for half in range(2):
                        idx = qc * 2 + half
                        otp = otps[half]
                        nc.vector.tensor_copy(otsb[hp][:, idx, :], otp[0:HD, :])
                        nc.vector.tensor_copy(sums_sb[hp][:, idx, :],
                                              otp[HD:HD + 1, :])
                        nc.sync.dma_start(sums_dram[hp, idx, :],
                                          sums_sb[hp][:, idx, :])

            def normalize_direct(hp, qc):
                # low-latency path: per-group DVE reciprocal + gpsimd broadcast
                for half in range(2):
                    idx = qc * 2 + half
                    rcd = ssm.tile([1, NQ], F32, tag="rcd",
                                   name=f"rcd_{hp}_{qc}_{half}")
                    srow2 = ssm.tile([1, NQ], F32, tag="ss2",
                                     name=f"ss2_{hp}_{qc}_{half}")
                    nc.vector.tensor_copy(srow2[:], sums_sb[hp][:, idx, :])
                    nc.vector.reciprocal(rcd[:], srow2[:])
                    bc = bcp.tile([HD, NQ], F32, tag="bc",
                                  name=f"bcd_{hp}_{qc}_{half}")
                    nc.gpsimd.partition_broadcast(bc[:], rcd[:])
                    nc.vector.tensor_tensor(
                        attnT_sb[half * HD:(half + 1) * HD, hp,
                                 qc * NQ:(qc + 1) * NQ],
                        otsb[hp][:, idx, :], bc[:], MUL)

            def normalize(hp, qcs=None):
                qcs = list(range(NT)) if qcs is None else qcs
                i0, i1 = qcs[0] * 2, qcs[-1] * 2 + 2
                coll = clp.tile([i1 - i0, NQ], F32, tag="cl",
                                name=f"cl_{hp}_{i0}")
                nc.sync.dma_start(coll[:], sums_dram[hp, i0:i1])
                rec = clp.tile([i1 - i0, NQ], F32, tag="rc", name=f"rc_{hp}_{i0}")
                nc.vector.reciprocal(rec[:], coll[:])
                nc.sync.dma_start(recip_dram[hp, i0:i1], rec[:])
                for qc in qcs:
                    for half in range(2):
                        idx = qc * 2 + half
                        bc = bcp.tile([HD, NQ], F32, tag="bc",
                                      name=f"bc_{hp}_{qc}_{half}")
                        row = recip_dram[hp, idx]
                        src = bass.AP(tensor=row.tensor, offset=row.offset,
                                      ap=[[0, HD]] + list(row.ap))
                        nc.sync.dma_start(bc[:], src)
                        nc.vector.tensor_tensor(
                            attnT_sb[half * HD:(half + 1) * HD, hp,
                                     qc * NQ:(qc + 1) * NQ],
                            otsb[hp][:, idx, :], bc[:], MUL)

            def wo_proj(iis=None):
                for i in (range(NKB) if iis is None else iis):
                    for j in range(D // NQ):
                        ps = st_ps_pool.tile([P, NQ], F32, tag="st", name=f"o_{i}_{j}")
                        for cb in range(CB):
                            nc.tensor.matmul(
                                ps[:],
                                attnT_sb[:, cb, i * P:(i + 1) * P],
                                woT_sb[:, cb, j * NQ:(j + 1) * NQ],
                                start=(cb == 0), stop=(cb == CB - 1))
                        ob = obp.tile([P, NQ], F32, tag="ob", name=f"ob_{i}_{j}")
                        nc.vector.tensor_copy(ob[:], ps[:])
                        nc.sync.dma_start(
                            out[i * P:(i + 1) * P, j * NQ:(j + 1) * NQ], ob[:])

            # emission order chosen so the PE always has projection work to fill
            # gaps while the scalar engine runs exp for the other head-pair
            project_T(wqT_sb, QT_sb, 0, "q")
            project_T(wkT_sb, KT_sb, 0, "k")
            project_v()
            attention(0)
            project_T(wqT_sb, QT_sb, 1, "q")
            project_T(wkT_sb, KT_sb, 1, "k")
            normalize(0)
            for qc in range(NT):
                attention(1, [qc])
                if qc >= 2:
                    normalize_direct(1, qc)
                else:
                    normalize(1, [qc])
                wo_proj(range(qc * 4, qc * 4 + 4))

    nc.compile()
    return nc


def _get_program(causal: bool):
    key = ("causal" if causal else "full")
    if key not in _prog_cache:
        _prog_cache[key] = _build_program(causal)
    return _prog_cache[key]


def _mask_kind(mask):
    m = np.asarray(mask)
    if m.ndim == 4:
        m = m[0, 0]
    if (m != 0).all():
        return False  # full attention
    trilm = np.tril(np.ones((m.shape[0], m.shape[1]), dtype=m.dtype))
    if np.array_equal(m, trilm):
        return True
    raise NotImplementedError("mask is neither all-ones nor causal tril")


def _make_in_maps(x, cos, sin, wq, wk, wv, wo):
    x = np.asarray(x, dtype=np.float32)
    cos = np.asarray(cos, dtype=np.float32)
    sin = np.asarray(sin, dtype=np.float32)
    wq = np.asarray(wq, dtype=np.float32)
    wk = np.asarray(wk, dtype=np.float32)
    wv = np.asarray(wv, dtype=np.float32)
    wo = np.asarray(wo, dtype=np.float32)

    # RoPE tables in transposed head-pair layout [128ch, T].
    # cos2T[c, t] = cos[t, c % 64]; sinsgn flips sign on the low half of each head;
    # sinS is additionally row-swapped (c ^ 32) so the shifted multiply can read
    # both inputs from the same base partition.
    ci = np.arange(P) % HD
    cos2T = np.ascontiguousarray(cos[:T, ci].T)               # [128, T]
    sgn = np.where((np.arange(P) % HD) < (HD // 2), -1.0, 1.0).astype(np.float32)
    sinsgn = sin[:T, ci].T * sgn[:, None]                      # [128, T]
    sinS = np.ascontiguousarray(
        sinsgn[np.arange(P) ^ 32, :].astype(np.float16))       # row-swapped
    trim = np.ascontiguousarray(
        np.triu(np.ones((P, P), dtype=np.float16)))            # allowed: k <= q
    ones = np.ones((P, (T // P) * HPC), dtype=np.float16)

    in_maps = []
    for core in range(NCORES):
        b = core // GROUPS
        g = core % GROUPS
        c0 = g * CH
        in_maps.append({
            "xT": np.ascontiguousarray(x[b].T.astype(np.float16)),          # [D, T]
            "wqT": np.ascontiguousarray(wq[c0:c0 + CH, :].T.astype(np.float16)),
            "wkT": np.ascontiguousarray(wk[c0:c0 + CH, :].T.astype(np.float16)),
            "wvT": np.ascontiguousarray(wv[c0:c0 + CH, :].T.astype(np.float16)),
            "woT": np.ascontiguousarray(wo[:, c0:c0 + CH].T.astype(np.float16)),
            "cosT": cos2T,
            "sinS": sinS,
            "tri": trim,
            "onescol": ones,
        })
    return in_maps


def _run(inputs, trace=False):
    from concourse import bass_utils
    causal = _mask_kind(inputs["mask"])
    nc = _get_program(causal)
    in_maps = _make_in_maps(
        inputs["x"], inputs["cos"], inputs["sin"],
        inputs["wq"], inputs["wk"], inputs["wv"], inputs["wo"])
    if trace:
        _install_ntff_shim()
    res = bass_utils.run_bass_kernel_spmd(
        nc, in_maps, core_ids=list(range(NCORES)), trace=trace)
    outs = [r["out"] for r in res.results]
    full = np.empty((B, T, D), dtype=np.float32)
    for b in range(B):
        full[b] = outs[b * GROUPS]
        for g in range(1, GROUPS):
            full[b] += outs[b * GROUPS + g]
    return full, res


def kernel(**inputs):
    full, _ = _run(inputs, trace=False)
    return full


def kernel_profiled(**inputs):
    """Like kernel() but with NTFF tracing; returns (out, BassKernelResults)."""
    return _run(inputs, trace=True)
